# revision 1
# baseline (speedup 1.0000x reference)
"""BinaryLinear (straight-through sign(w)) kernel for Trainium2, 8 NeuronCores.

Computes out = x @ sign(w).T + b for
  x: [8192, 2048] f32, w: [4096, 2048] f32, b: [4096] f32 -> out [8192, 4096] f32.

Sharding: 4-way data parallel (batch) x 2-way tensor parallel (out_features).
Each core computes a [2048, 2048] block of the output:
  out[bi*2048:(bi+1)*2048, fi*2048:(fi+1)*2048]
    = x_shard @ sign(w_shard).T + b_shard.

Per-core device kernel (fp16 matmul, fp32 accumulate — fp16 runs at the same
PE rate as bf16 but keeps 10 mantissa bits; sign(w) in {-1,0,1} is exact):
  - the whole w^T shard [2048, 2048] fp16 lives in SBUF (64 KiB/partition),
    loaded once;
  - x^T tiles stream through a multi-buffered pool;
  - bias is added during the PSUM->SBUF copyback on the vector engine.
"""

from contextlib import ExitStack

import numpy as np

# Full problem shapes (hardcoded per the grading contract).
M, K, N = 8192, 2048, 4096
P_BATCH, P_FEAT = 4, 2  # 4 x 2 core grid
MC, NC = M // P_BATCH, N // P_FEAT  # 2048, 2048 per-core block
N_CORES = P_BATCH * P_FEAT
P = 128


def build_nc(mc: int = MC, k: int = K, nc_dim: int = NC, reps: int = 1):
    """Build + compile the per-core Bass module: out[mc, nc_dim] = xt^T @ wt + bias.

    reps > 1 repeats the whole computation (for slope-based benchmarking)."""
    import concourse.mybir as mybir
    import concourse.tile as tile
    from concourse import bacc
    from concourse.bass import ts
    from concourse.kernels.tile_matmul import (
        ShapeInfo,
        composable_matmul_tile_kernel,
    )

    ko = k // P
    MAX_K_TILE = 512
    k_tile = min(MAX_K_TILE, k)
    k_tiles = k // k_tile
    k_subtiles = k_tile // P
    TB = 512  # m/n tile width of the pre-blocked host layouts
    m_tiles = mc // TB
    n_blocks = nc_dim // TB

    nc = bacc.Bacc("TRN2", target_bir_lowering=False, debug=False)
    # Inputs arrive pre-blocked on the host (see _pack_blocks): each
    # [P, k_subtiles, TB] block is fully contiguous in DRAM, so every DMA has
    # 4-KiB-per-partition descriptor runs instead of 1-KiB strided ones.
    xt = nc.dram_tensor(
        "xt", [m_tiles, k_tiles, P, k_subtiles, TB], mybir.dt.float16,
        kind="ExternalInput",
    )
    # w uses an n-block-major layout ([nb, p, ko, n]) so each n-block's
    # preload is contiguous per partition on BOTH sides: 16-KiB descriptor
    # runs instead of 4-KiB, 4x fewer descriptors for the Q7 to emit.
    wt = nc.dram_tensor(
        "wt", [n_blocks, P, ko, TB], mybir.dt.float16, kind="ExternalInput"
    )
    bias = nc.dram_tensor("bias", [nc_dim], mybir.dt.float32, kind="ExternalInput")
    out = nc.dram_tensor("out", [mc, nc_dim], mybir.dt.float32, kind="ExternalOutput")

    with tile.TileContext(nc) as tc, ExitStack() as ctx:
        # HAM warmup: the PE clock is gated to 1.2 GHz until ~3.4 us of
        # sustained activity. The first real matmuls can't start until their
        # operands arrive (~5 us of DMA ramp), so spend the idle window on
        # throwaway matmuls over a zeroed scratch tile — the cold-clock
        # penalty lands on them instead of the real work. The scratch SBUF
        # pool stays OPEN so its slot is never reused (a close would order
        # the w preload behind the dummy reads); only the PSUM bank is
        # returned before the real kernel needs all 8.
        warm_sb = ctx.enter_context(tc.tile_pool(name="warm_sb", bufs=1))
        scratch = warm_sb.tile([P, 512], mybir.dt.float16)
        nc.vector.memset(scratch[:], 0.0)
        with tc.tile_pool(name="warm_ps", bufs=1, space="PSUM") as wps_pool:
            ps = wps_pool.tile([P, 512], mybir.dt.float32)
            for _ in range(10):
                nc.tensor.matmul(
                    ps[:], scratch[:, :P], scratch[:], start=True, stop=True
                )

        const = ctx.enter_context(tc.tile_pool(name="const", bufs=1))
        kxm_pool = ctx.enter_context(tc.tile_pool(name="kxm", bufs=k_tiles + 1))

        # Whole w^T shard resident in SBUF, n-block-major [p, nb, ko, n] with
        # cache[p, nb, o, j] = w^T[o*128 + p, nb*TB + j]. Preload runs on the
        # gpsimd (SWDGE) queue so the x-tile loads (HWDGE via nc.sync) are not
        # serialized behind it, in n-major order: the first output tile
        # consumes (n0, k0..k3), so all its chunks must land first.
        w_sb = const.tile([P, n_blocks, ko, TB], mybir.dt.float16)
        for nb in range(n_blocks):
            if nb == 0:
                # first block split per k-tile so the first matmuls unblock
                # at k-tile granularity
                for kt in range(k_tiles):
                    sl = slice(kt * k_subtiles, (kt + 1) * k_subtiles)
                    nc.gpsimd.dma_start(
                        out=w_sb[:, 0, sl, :], in_=wt.ap()[0, :, sl, :]
                    )
            else:
                nc.gpsimd.dma_start(out=w_sb[:, nb], in_=wt.ap()[nb])

        # Bias replicated across all 128 partitions so the copyback can add the
        # n-slice with a plain tensor_tensor add. One tiny [1, N] HBM read on
        # the otherwise-idle ACT HWDGE ring + an on-chip partition broadcast —
        # a [128, N] broadcast DMA on the SWDGE queue behind the w preload
        # would block the first evictions (and PSUM recycling) until ~36 us.
        bias_sb = const.tile([P, nc_dim], mybir.dt.float32)
        nc.scalar.dma_start(out=bias_sb[:1, :], in_=bias.ap()[None, :])
        nc.gpsimd.partition_broadcast(bias_sb[:], bias_sb[:1, :])

        # Custom kxm producer: one contiguous-block DMA per k-tile of x^T.
        def kxm_producer(nc_, md):
            t = kxm_pool.tile([P, md.k_subtiles, md.m_tile], mybir.dt.float16, tag="kxm")
            if md.m_tile_idx == 1 and n_blocks > 2:
                # Ordering-only dep: m1's prefetch is not needed until ~60 us
                # but otherwise jumps the shared DMA mover ahead of the w n1/n2
                # blocks (needed at ~20/~33 us). A tiny read of the w n2 region
                # into this tile makes the real load schedule after that w
                # block has transferred.
                nc_.vector.tensor_copy(out=t[:1, :1, :2], in_=w_sb[:1, 2, :1, :2])
            nc_.sync.dma_start(out=t[:], in_=xt.ap()[md.m_tile_idx, md.k_tile_idx])
            return t

        kxm_shape = ShapeInfo(pdims=((P, ko),), fdims=(mc,))

        def kxn_producer(nc_, md):
            return w_sb[:, md.n_tile_idx, ts(md.k_tile_idx, md.k_subtiles), :]

        kxn_shape = ShapeInfo(pdims=((P, ko),), fdims=(nc_dim,))

        out_t = out.ap().rearrange("(o p) n -> p o n", p=P)

        def add_bias_store_reducer(nc_, psum, sbuf, md):
            # psum -> sbuf with the bias added, then store this subtile
            # immediately (finer-grained than the stock whole-tile consumer,
            # so stores overlap the remaining evictions and the tail drains
            # faster).
            sz = md.n_subtile_slice_size
            nc_.vector.tensor_add(
                out=sbuf[:, :, :sz],
                in0=psum[:, :sz],
                in1=bias_sb[: psum.shape[0], md.n_subtile_slice],
            )
            po = md.m_tile_idx * md.m_subtiles + md.m_subtile_idx
            nc_.sync.dma_start(
                out=out_t[:, po : po + 1, md.n_subtile_slice], in_=sbuf[:, :, :sz]
            )

        for _ in range(reps):
            composable_matmul_tile_kernel(
                tc=tc,
                kxm_shape=kxm_shape,
                kxn_shape=kxn_shape,
                output_type=mybir.dt.float32,
                kxm_producer=kxm_producer,
                kxn_producer=kxn_producer,
                mxn_consumer=lambda nc_, tile_, md: None,
                mxn_subtile_reducer=add_bias_store_reducer,
                MAX_K_TILE_SIZE=MAX_K_TILE,
                psum_n_bufs=2,
            )

    nc.compile()
    return nc


def _pack_w_nblocks(a: np.ndarray, tb: int = 512) -> np.ndarray:
    """[N, K] row-major -> [N//tb, 128, K//128, tb] with
    block[nb, p, o, j] = a[nb*tb + j, o*128 + p]; per-partition-contiguous
    [ko, tb] planes -> 16-KiB DMA descriptor runs."""
    n, k = a.shape
    v = a.reshape(n // tb, tb, k // P, P)
    return np.ascontiguousarray(v.transpose(0, 3, 2, 1))


def _pack_blocks(a: np.ndarray, tb: int = 512) -> np.ndarray:
    """[F, K] row-major -> [F//tb, K//ktw, 128, ks, tb] DMA-contiguous blocks.

    block[ft, kt, p, s, j] = a[ft*tb + j, kt*ktw + s*128 + p], i.e. each
    [128, ks, tb] block is one fully-contiguous DMA source with K on the
    partition dim (a^T layout within the block)."""
    f, k = a.shape
    ktw = min(512, k)
    kts, ks = k // ktw, ktw // P
    v = a.reshape(f // tb, tb, kts, ks, P)
    return np.ascontiguousarray(v.transpose(0, 2, 4, 3, 1))


_NC_CACHE = None


def _get_nc():
    global _NC_CACHE
    if _NC_CACHE is None:
        _NC_CACHE = build_nc()
    return _NC_CACHE


def kernel(x: np.ndarray, w: np.ndarray, b: np.ndarray) -> np.ndarray:
    from concourse.bass_utils import run_bass_kernel_spmd

    x = np.asarray(x, dtype=np.float32)
    w = np.asarray(w, dtype=np.float32)
    b = np.asarray(b, dtype=np.float32)

    f16 = np.float16
    x_f16 = x.astype(f16)
    w_f16 = np.sign(w).astype(f16)

    # Unique DMA-blocked shards (x per batch group, sign(w) per feature
    # group), packed in parallel (numpy releases the GIL on these copies).
    from concurrent.futures import ThreadPoolExecutor

    with ThreadPoolExecutor(max_workers=6) as pool:
        xt_f = [
            pool.submit(_pack_blocks, x_f16[bi * MC : (bi + 1) * MC])
            for bi in range(P_BATCH)
        ]
        wt_f = [
            pool.submit(_pack_w_nblocks, w_f16[fi * NC : (fi + 1) * NC])
            for fi in range(P_FEAT)
        ]
        xt_shards = [f.result() for f in xt_f]
        wt_shards = [f.result() for f in wt_f]
    b_shards = [np.ascontiguousarray(b[fi * NC : (fi + 1) * NC]) for fi in range(P_FEAT)]

    in_maps = []
    for c in range(N_CORES):
        bi, fi = divmod(c, P_FEAT)
        in_maps.append(
            {"xt": xt_shards[bi], "wt": wt_shards[fi], "bias": b_shards[fi]}
        )

    nc = _get_nc()
    try:
        results = run_bass_kernel_spmd(
            nc, in_maps, core_ids=list(range(N_CORES))
        ).results
    except Exception:
        # One retry for transient runtime/relay failures.
        results = run_bass_kernel_spmd(
            nc, in_maps, core_ids=list(range(N_CORES))
        ).results

    out = np.empty((M, N), dtype=np.float32)
    for c in range(N_CORES):
        bi, fi = divmod(c, P_FEAT)
        out[bi * MC : (bi + 1) * MC, fi * NC : (fi + 1) * NC] = results[c]["out"]
    return out



# revision 2
# speedup vs baseline: 1.5080x; 1.5080x over previous
"""BinaryLinear (straight-through sign(w)) kernel for Trainium2, 8 NeuronCores.

Computes out = x @ sign(w).T + b for
  x: [8192, 2048] f32, w: [4096, 2048] f32, b: [4096] f32 -> out [8192, 4096] f32.

Sharding: 4-way data parallel (batch) x 2-way tensor parallel (out_features).
Each core computes a [2048, 2048] block of the output:
  out[bi*2048:(bi+1)*2048, fi*2048:(fi+1)*2048]
    = x_shard @ sign(w_shard).T + b_shard.

Per-core device kernel — mixed-precision contraction, fp32 accumulate:
  - K is split K8 + K16.  The first K8 in_features use fp8e4 (e4m3) operands
    with perf_mode=DoubleRow (2 fp8 weights per PE cell, 256-deep virtual
    contraction): 2x the PE FLOP rate of fp16.  sign(w) in {-1,0,1} is exact
    in e4m3; only x pays quantization error (~2.65% rms per column, measured
    on the fixed problem data), diluted to 2.65%*sqrt(K8/2048) of the output.
    K8=1024 -> 1.87% rel l2, inside the 2e-2 budget.  The remaining K16
    columns run in fp16 (exact to ~3e-4).
  - whole sign(w)^T shard lives in SBUF (fp8 + fp16 planes), loaded once;
  - x^T tiles stream through multi-buffered pools (fp8 and fp16);
  - bias is added during the PSUM->SBUF copyback on the vector engine.
"""

from contextlib import ExitStack

import numpy as np

# Full problem shapes (hardcoded per the grading contract).
M, K, N = 8192, 2048, 4096
P_BATCH, P_FEAT = 4, 2  # 4 x 2 core grid
MC, NC = M // P_BATCH, N // P_FEAT  # 2048, 2048 per-core block
N_CORES = P_BATCH * P_FEAT
P = 128
K8 = 1024  # in_features contracted in fp8e4 DoubleRow (multiple of 512)
K16 = K - K8  # in_features contracted in fp16


def build_nc(mc: int = MC, nc_dim: int = NC, reps: int = 1):
    """Build + compile the per-core Bass module:
    out[mc, nc_dim] = x8^T.T @ w8 + x16^T.T @ w16 + bias.

    reps > 1 repeats the whole computation (for slope-based benchmarking)."""
    import concourse.mybir as mybir
    import concourse.tile as tile
    from concourse import bacc
    from concourse.bass import ts
    from concourse.kernels.tile_matmul import (
        ShapeInfo,
        composable_matmul_tile_kernel,
    )

    TB = 512  # m/n tile width of the pre-blocked host layouts
    KT = 512  # k-tile width
    KS = KT // P  # k-subtiles per k-tile (4)
    ko8, ko16 = K8 // P, K16 // P
    k8_tiles, k16_tiles = K8 // KT, K16 // KT
    m_tiles = mc // TB
    n_blocks = nc_dim // TB

    nc = bacc.Bacc("TRN2", target_bir_lowering=False, debug=False)
    # x inputs arrive pre-blocked on the host (see _pack_blocks): each
    # [P, KS, TB] block is fully contiguous in DRAM, so every DMA has
    # large per-partition descriptor runs instead of strided ones.
    xt8 = nc.dram_tensor(
        "xt8", [m_tiles, k8_tiles, P, KS, TB], mybir.dt.float8e4,
        kind="ExternalInput",
    )
    xt16 = nc.dram_tensor(
        "xt16", [m_tiles, k16_tiles, P, KS, TB], mybir.dt.float16,
        kind="ExternalInput",
    )
    # w uses an n-block-major layout ([nb, p, ko, n]) so each n-block's
    # preload is contiguous per partition on BOTH sides.
    wt8 = nc.dram_tensor(
        "wt8", [n_blocks, P, ko8, TB], mybir.dt.float8e4, kind="ExternalInput"
    )
    wt16 = nc.dram_tensor(
        "wt16", [n_blocks, P, ko16, TB], mybir.dt.float16, kind="ExternalInput"
    )
    bias = nc.dram_tensor("bias", [nc_dim], mybir.dt.float32, kind="ExternalInput")
    out = nc.dram_tensor("out", [mc, nc_dim], mybir.dt.float32, kind="ExternalOutput")

    with tile.TileContext(nc) as tc, ExitStack() as ctx:
        # HAM warmup: the PE clock is gated to 1.2 GHz until ~3.4 us of
        # sustained activity. The first real matmuls can't start until their
        # operands arrive (~5 us of DMA ramp), so spend the idle window on
        # throwaway matmuls over a zeroed scratch tile — the cold-clock
        # penalty lands on them instead of the real work. The scratch SBUF
        # pool stays OPEN so its slot is never reused (a close would order
        # the w preload behind the dummy reads); only the PSUM bank is
        # returned before the real kernel needs all 8.
        warm_sb = ctx.enter_context(tc.tile_pool(name="warm_sb", bufs=1))
        scratch = warm_sb.tile([P, 512], mybir.dt.float16)
        nc.vector.memset(scratch[:], 0.0)
        with tc.tile_pool(name="warm_ps", bufs=1, space="PSUM") as wps_pool:
            ps = wps_pool.tile([P, 512], mybir.dt.float32)
            for _ in range(10):
                nc.tensor.matmul(
                    ps[:], scratch[:, :P], scratch[:], start=True, stop=True
                )

        const = ctx.enter_context(tc.tile_pool(name="const", bufs=1))
        kxm8_pool = ctx.enter_context(tc.tile_pool(name="kxm8", bufs=k8_tiles + 1))
        kxm16_pool = ctx.enter_context(tc.tile_pool(name="kxm16", bufs=k16_tiles + 1))

        # Whole sign(w)^T shard resident in SBUF, n-block-major, as an fp8
        # plane (first K8 in_features) and an fp16 plane (rest):
        #   w8_sb[p, nb, o, j]  = sign(w)^T[o*128 + p, nb*TB + j],  o <  ko8
        #   w16_sb[p, nb, o, j] = sign(w)^T[K8 + o*128 + p, nb*TB + j]
        # Preload runs on the gpsimd (SWDGE) queue so the x-tile loads
        # (HWDGE via nc.sync) are not serialized behind it, in n-major order:
        # the first output tile consumes (n0, k0..), so its chunks land first.
        w8_sb = const.tile([P, n_blocks, ko8, TB], mybir.dt.float8e4)
        w16_sb = const.tile([P, n_blocks, ko16, TB], mybir.dt.float16)
        for nb in range(n_blocks):
            if nb == 0:
                # first block split per k-tile so the first matmuls unblock
                # at k-tile granularity
                for kt in range(k8_tiles):
                    sl = slice(kt * KS, (kt + 1) * KS)
                    nc.gpsimd.dma_start(
                        out=w8_sb[:, 0, sl, :], in_=wt8.ap()[0, :, sl, :]
                    )
                for kt in range(k16_tiles):
                    sl = slice(kt * KS, (kt + 1) * KS)
                    nc.gpsimd.dma_start(
                        out=w16_sb[:, 0, sl, :], in_=wt16.ap()[0, :, sl, :]
                    )
            else:
                nc.gpsimd.dma_start(out=w8_sb[:, nb], in_=wt8.ap()[nb])
                nc.gpsimd.dma_start(out=w16_sb[:, nb], in_=wt16.ap()[nb])

        # Bias replicated across all 128 partitions so the copyback can add the
        # n-slice with a plain tensor_tensor add. One tiny [1, N] HBM read on
        # the otherwise-idle ACT HWDGE ring + an on-chip partition broadcast.
        bias_sb = const.tile([P, nc_dim], mybir.dt.float32)
        nc.scalar.dma_start(out=bias_sb[:1, :], in_=bias.ap()[None, :])
        nc.gpsimd.partition_broadcast(bias_sb[:], bias_sb[:1, :])

        # Custom kxm producer: one contiguous-block DMA per k-tile of x^T,
        # fp8 pool for k-batch 0, fp16 pool for k-batch 1.
        def kxm_producer(nc_, md):
            if md.k_batch_idx == 0:
                t = kxm8_pool.tile([P, KS, md.m_tile], mybir.dt.float8e4, tag="kxm8")
                src = xt8.ap()[md.m_tile_idx, md.k_tile_idx]
            else:
                t = kxm16_pool.tile([P, KS, md.m_tile], mybir.dt.float16, tag="kxm16")
                src = xt16.ap()[md.m_tile_idx, md.k_tile_idx]
            if md.k_batch_idx == 0 and md.k_tile_idx == 0 and md.m_tile_idx == 1 \
                    and n_blocks > 2:
                # Ordering-only dep: m1's prefetch is not needed until much
                # later but otherwise jumps the shared DMA mover ahead of the
                # later w n-blocks. A tiny read of the w16 n2 region makes the
                # real load schedule after that w block has transferred.
                nc_.vector.tensor_copy(out=t[:1, :1, :2], in_=w16_sb[:1, 2, :1, :2])
            nc_.sync.dma_start(out=t[:], in_=src)
            return t

        kxm_shape = ShapeInfo(pdims=((P, ko8), (P, ko16)), fdims=(mc,))

        def kxn_producer(nc_, md):
            if md.k_batch_idx == 0:
                return w8_sb[:, md.n_tile_idx, ts(md.k_tile_idx, KS), :]
            return w16_sb[:, md.n_tile_idx, ts(md.k_tile_idx, KS), :]

        kxn_shape = ShapeInfo(pdims=((P, ko8), (P, ko16)), fdims=(nc_dim,))

        out_t = out.ap().rearrange("(o p) n -> p o n", p=P)

        def add_bias_store_reducer(nc_, psum, sbuf, md):
            # psum -> sbuf with the bias added, then store this subtile
            # immediately (finer-grained than the stock whole-tile consumer,
            # so stores overlap the remaining evictions and the tail drains
            # faster).
            sz = md.n_subtile_slice_size
            nc_.vector.tensor_add(
                out=sbuf[:, :, :sz],
                in0=psum[:, :sz],
                in1=bias_sb[: psum.shape[0], md.n_subtile_slice],
            )
            po = md.m_tile_idx * md.m_subtiles + md.m_subtile_idx
            nc_.sync.dma_start(
                out=out_t[:, po : po + 1, md.n_subtile_slice], in_=sbuf[:, :, :sz]
            )

        for _ in range(reps):
            composable_matmul_tile_kernel(
                tc=tc,
                kxm_shape=kxm_shape,
                kxn_shape=kxn_shape,
                output_type=mybir.dt.float32,
                kxm_producer=kxm_producer,
                kxn_producer=kxn_producer,
                mxn_consumer=lambda nc_, tile_, md: None,
                mxn_subtile_reducer=add_bias_store_reducer,
                MAX_K_TILE_SIZE=KT,
                psum_n_bufs=2,
            )

    nc.compile()
    return nc


def _pack_w_nblocks(a: np.ndarray, tb: int = 512) -> np.ndarray:
    """[N, K] row-major -> [N//tb, 128, K//128, tb] with
    block[nb, p, o, j] = a[nb*tb + j, o*128 + p]; per-partition-contiguous
    [ko, tb] planes -> large DMA descriptor runs."""
    n, k = a.shape
    v = a.reshape(n // tb, tb, k // P, P)
    return np.ascontiguousarray(v.transpose(0, 3, 2, 1))


def _pack_blocks(a: np.ndarray, tb: int = 512) -> np.ndarray:
    """[F, K] row-major -> [F//tb, K//ktw, 128, ks, tb] DMA-contiguous blocks.

    block[ft, kt, p, s, j] = a[ft*tb + j, kt*ktw + s*128 + p], i.e. each
    [128, ks, tb] block is one fully-contiguous DMA source with K on the
    partition dim (a^T layout within the block)."""
    f, k = a.shape
    ktw = min(512, k)
    kts, ks = k // ktw, ktw // P
    v = a.reshape(f // tb, tb, kts, ks, P)
    return np.ascontiguousarray(v.transpose(0, 2, 4, 3, 1))


_NC_CACHE = None


def _get_nc():
    global _NC_CACHE
    if _NC_CACHE is None:
        _NC_CACHE = build_nc()
    return _NC_CACHE


def kernel(x: np.ndarray, w: np.ndarray, b: np.ndarray) -> np.ndarray:
    import ml_dtypes
    from concourse.bass_utils import run_bass_kernel_spmd

    x = np.asarray(x, dtype=np.float32)
    w = np.asarray(w, dtype=np.float32)
    b = np.asarray(b, dtype=np.float32)

    f8 = ml_dtypes.float8_e4m3
    f16 = np.float16
    s = np.sign(w)

    # Unique DMA-blocked shards (x per batch group, sign(w) per feature
    # group), packed in parallel (numpy releases the GIL on these copies).
    from concurrent.futures import ThreadPoolExecutor

    def pack_x8(bi):
        return _pack_blocks(x[bi * MC : (bi + 1) * MC, :K8].astype(f8))

    def pack_x16(bi):
        return _pack_blocks(x[bi * MC : (bi + 1) * MC, K8:].astype(f16))

    def pack_w8(fi):
        return _pack_w_nblocks(s[fi * NC : (fi + 1) * NC, :K8].astype(f8))

    def pack_w16(fi):
        return _pack_w_nblocks(s[fi * NC : (fi + 1) * NC, K8:].astype(f16))

    with ThreadPoolExecutor(max_workers=8) as pool:
        x8_f = [pool.submit(pack_x8, bi) for bi in range(P_BATCH)]
        x16_f = [pool.submit(pack_x16, bi) for bi in range(P_BATCH)]
        w8_f = [pool.submit(pack_w8, fi) for fi in range(P_FEAT)]
        w16_f = [pool.submit(pack_w16, fi) for fi in range(P_FEAT)]
        x8_shards = [f.result() for f in x8_f]
        x16_shards = [f.result() for f in x16_f]
        w8_shards = [f.result() for f in w8_f]
        w16_shards = [f.result() for f in w16_f]
    b_shards = [np.ascontiguousarray(b[fi * NC : (fi + 1) * NC]) for fi in range(P_FEAT)]

    in_maps = []
    for c in range(N_CORES):
        bi, fi = divmod(c, P_FEAT)
        in_maps.append(
            {
                "xt8": x8_shards[bi],
                "xt16": x16_shards[bi],
                "wt8": w8_shards[fi],
                "wt16": w16_shards[fi],
                "bias": b_shards[fi],
            }
        )

    nc = _get_nc()
    try:
        results = run_bass_kernel_spmd(
            nc, in_maps, core_ids=list(range(N_CORES))
        ).results
    except Exception:
        # One retry for transient runtime/relay failures.
        results = run_bass_kernel_spmd(
            nc, in_maps, core_ids=list(range(N_CORES))
        ).results

    out = np.empty((M, N), dtype=np.float32)
    for c in range(N_CORES):
        bi, fi = divmod(c, P_FEAT)
        out[bi * MC : (bi + 1) * MC, fi * NC : (fi + 1) * NC] = results[c]["out"]
    return out


# revision 17
# speedup vs baseline: 1.5417x; 1.0223x over previous
"""BinaryLinear (straight-through sign(w)) kernel for Trainium2, 8 NeuronCores.

Computes out = x @ sign(w).T + b for
  x: [8192, 2048] f32, w: [4096, 2048] f32, b: [4096] f32 -> out [8192, 4096] f32.

Sharding: 4-way data parallel (batch) x 2-way tensor parallel (out_features).
Each core computes a [2048, 2048] block of the output:
  out[bi*2048:(bi+1)*2048, fi*2048:(fi+1)*2048]
    = x_shard @ sign(w_shard).T + b_shard.

Per-core device kernel — mixed-precision contraction, fp32 accumulate:
  - K is split K8 + K16.  The first K8 in_features use fp8e4 (e4m3) operands
    with perf_mode=DoubleRow (2 fp8 weights per PE cell, 256-deep virtual
    contraction): 2x the PE FLOP rate of fp16.  sign(w) in {-1,0,1} is exact
    in e4m3; only x pays quantization error (~2.65% rms per element, measured
    on the fixed problem data), diluted to 2.65%*sqrt(K8/2048) of the output.
    K8=1024 -> 1.87% rel l2, inside the 2e-2 budget.  The remaining K16
    columns run in fp16 (exact to ~3e-4).
  - whole sign(w)^T shard lives in SBUF (fp8 + fp16 planes), loaded once,
    split across the ACT HWDGE ring (first n-blocks) and the gpsimd SWDGE
    queue (rest) so the first matmuls and the steady stream are both fed;
  - x^T tiles stream through multi-buffered pools on the SP HWDGE ring,
    with the next m-tile prefetched two n-blocks early;
  - PSUM accumulation is m-subtile-major: each [128, 512] output tile runs
    its whole K reduction back-to-back, so evictions stagger and the tail
    after the last matmul is one eviction + one store instead of four;
  - bias is added during the PSUM->SBUF copyback on the vector engine.
"""

from contextlib import ExitStack

import numpy as np

# Full problem shapes (hardcoded per the grading contract).
M, K, N = 8192, 2048, 4096
P_BATCH, P_FEAT = 4, 2  # 4 x 2 core grid
MC, NC = M // P_BATCH, N // P_FEAT  # 2048, 2048 per-core block
N_CORES = P_BATCH * P_FEAT
P = 128
K8 = 1024  # in_features contracted in fp8e4 DoubleRow (multiple of 512)
K16 = K - K8  # in_features contracted in fp16
N_WARM = 5  # HAM warmup matmuls
BIAS_EARLY = True  # bias row before the w16 n0 chunks in the ACT queue
TAIL_SPLIT = True  # halve the final eviction+store chain


def build_nc(mc: int = MC, nc_dim: int = NC, reps: int = 1):
    """Build + compile the per-core Bass module:
    out[mc, nc_dim] = x8^T.T @ w8 + x16^T.T @ w16 + bias.

    reps > 1 repeats the whole computation (for slope-based benchmarking)."""
    import concourse.mybir as mybir
    import concourse.tile as tile
    from concourse import bacc

    TB = 512  # m/n tile width of the pre-blocked host layouts
    KT = 512  # k-tile width
    KS = KT // P  # k-subtiles per k-tile (4)
    ko8, ko16 = K8 // P, K16 // P
    k8_tiles, k16_tiles = K8 // KT, K16 // KT
    m_tiles = mc // TB
    m_subs = TB // P  # m-subtiles per m-tile (4)
    n_blocks = nc_dim // TB

    nc = bacc.Bacc("TRN2", target_bir_lowering=False, debug=False)
    DR = mybir.MatmulPerfMode.DoubleRow

    # x inputs arrive pre-blocked on the host (see _pack_blocks): each
    # [P, KS, TB] block is fully contiguous in DRAM, so every DMA has
    # large per-partition descriptor runs instead of strided ones.
    xt8 = nc.dram_tensor(
        "xt8", [m_tiles, k8_tiles, P, KS, TB], mybir.dt.float8e4,
        kind="ExternalInput",
    )
    xt16 = nc.dram_tensor(
        "xt16", [m_tiles, k16_tiles, P, KS, TB], mybir.dt.float16,
        kind="ExternalInput",
    )
    # w uses an n-block-major layout ([nb, p, ko, n]) so each n-block's
    # preload is contiguous per partition on BOTH sides.  The whole sign
    # matrix ships as ONE fp8 tensor: the DoubleRow phase needs fp8 anyway,
    # and the fp16 phase uses the fp8 plane directly as its MOVING operand
    # (sign values are exact in e4m3; the PE upconverts per-operand, and the
    # matmul cost is set by the moving dtype at the same 1 row/cycle as
    # fp16) — 1 byte/weight instead of 2 through the startup DMA crunch.
    ko = ko8 + ko16
    wt = nc.dram_tensor(
        "wt", [n_blocks, P, ko, TB], mybir.dt.float8e4, kind="ExternalInput"
    )
    bias = nc.dram_tensor("bias", [nc_dim], mybir.dt.float32, kind="ExternalInput")
    out = nc.dram_tensor("out", [mc, nc_dim], mybir.dt.float32, kind="ExternalOutput")

    with tile.TileContext(nc) as tc, ExitStack() as ctx:
        # HAM warmup: the PE clock is gated (0.65/1.2 GHz) until ~3 us of
        # sustained activity. Start throwaway matmuls as early as possible
        # (gpsimd memset is ~100 ns) so the ramp burns while the first
        # operand DMAs are in flight. The scratch SBUF pool stays OPEN so
        # its slot is never reused; only the PSUM bank is returned.
        warm_sb = ctx.enter_context(tc.tile_pool(name="warm_sb", bufs=1))
        scratch = warm_sb.tile([P, 512], mybir.dt.float16)
        nc.vector.memset(scratch[:], 0.0)
        with tc.tile_pool(name="warm_ps", bufs=1, space="PSUM") as wps_pool:
            ps = wps_pool.tile([P, 512], mybir.dt.float32)
            for _ in range(N_WARM):
                nc.tensor.matmul(
                    ps[:], scratch[:, :P], scratch[:], start=True, stop=True
                )

        const = ctx.enter_context(tc.tile_pool(name="const", bufs=1))
        x8_pool = ctx.enter_context(tc.tile_pool(name="x8", bufs=2 * k8_tiles))
        x16_pool = ctx.enter_context(tc.tile_pool(name="x16", bufs=2 * k16_tiles))
        ev_pool = ctx.enter_context(tc.tile_pool(name="ev", bufs=4))
        psum = ctx.enter_context(tc.tile_pool(name="psum", bufs=6, space="PSUM"))

        # Whole sign(w)^T shard resident in SBUF, n-block-major, as an fp8
        # plane (first K8 in_features) and an fp16 plane (rest):
        #   w8_sb[p, nb, o, j]  = sign(w)^T[o*128 + p, nb*TB + j],  o <  ko8
        #   w16_sb[p, nb, o, j] = sign(w)^T[K8 + o*128 + p, nb*TB + j]
        w_sb = const.tile([P, n_blocks, ko, TB], mybir.dt.float8e4)

        bias_sb = const.tile([P, nc_dim], mybir.dt.float32)

        # The cost model serializes ALL DMA transfers on one shared pipe and
        # alternates HWDGE descriptor generation between the SP and ACT
        # queues; in-queue gens are FIFO, so transfer order tracks issue
        # order. The early transfer order is the startup critical path: n0
        # split per k-tile (its chunks interleave with the SP-ring x loads
        # of m0, matching the first block's consumption order), then the
        # bias row, then n1..n3 whole. SWDGE is avoided for w entirely — its
        # independent transfers would jump ahead of the critical n0 stream.
        # n0 in 4 k-tile chunks (matching the startup block's consumption
        # order against the SP-ring x loads), bias, then n1..n3 in
        # fp8-phase/fp16-phase halves so each matmul group's region dep
        # waits only on the half it reads.
        for kt in range(ko // KS):
            sl = slice(kt * KS, (kt + 1) * KS)
            if BIAS_EARLY and kt == 2:
                nc.scalar.dma_start(out=bias_sb[:1, :], in_=bias.ap()[None, :])
            nc.scalar.dma_start(out=w_sb[:, 0, sl, :], in_=wt.ap()[0, :, sl, :])
        if not BIAS_EARLY:
            nc.scalar.dma_start(out=bias_sb[:1, :], in_=bias.ap()[None, :])
        for nb in range(1, n_blocks):
            nc.scalar.dma_start(out=w_sb[:, nb, :ko8], in_=wt.ap()[nb, :, :ko8])
            nc.scalar.dma_start(out=w_sb[:, nb, ko8:], in_=wt.ap()[nb, :, ko8:])

        nc.gpsimd.partition_broadcast(bias_sb[:], bias_sb[:1, :])

        out_t = out.ap().rearrange("(o p) n -> p o n", p=P)

        def load_m(m):
            """Issue the x-tile loads for m-tile m on the SP HWDGE ring."""
            t8 = []
            for kt in range(k8_tiles):
                t = x8_pool.tile([P, KS, TB], mybir.dt.float8e4, tag="x8")
                nc.sync.dma_start(out=t[:], in_=xt8.ap()[m, kt])
                t8.append(t)
            t16 = []
            for kt in range(k16_tiles):
                t = x16_pool.tile([P, KS, TB], mybir.dt.float16, tag="x16")
                nc.sync.dma_start(out=t[:], in_=xt16.ap()[m, kt])
                t16.append(t)
            return t8, t16

        def mm_fp8(pt, x8_t, nb, kt, kk, sub, start):
            ms = slice(sub * P, (sub + 1) * P)
            nc.tensor.matmul(
                pt[:],
                x8_t[kt][:, 2 * kk : 2 * kk + 2, ms],
                w_sb[:, nb, kt * KS + 2 * kk : kt * KS + 2 * kk + 2, :],
                start=start,
                stop=False,
                perf_mode=DR,
            )

        def mm_fp16(pt, x16_t, nb, kt, s, sub, stop):
            ms = slice(sub * P, (sub + 1) * P)
            nc.tensor.matmul(
                pt[:],
                x16_t[kt][:, s, ms],
                w_sb[:, nb, ko8 + kt * KS + s, :],
                start=False,
                stop=stop,
            )

        def evict(pt, m, nb, sub):
            po = m * m_subs + sub
            last_block = m == m_tiles - 1 and nb == n_blocks - 1
            ev = ev_pool.tile([P, TB], mybir.dt.float32, tag="ev")
            if TAIL_SPLIT and last_block and sub == m_subs - 1:
                # Final tile: halve the add+store chain so the last HBM
                # write (whose completion latency ends the kernel) starts as
                # early as possible; the two store gens go to different
                # HWDGE rings so they don't serialize.
                for h, eng in ((0, nc.sync), (1, nc.scalar)):
                    cs = slice(h * (TB // 2), (h + 1) * (TB // 2))
                    nc.vector.tensor_add(
                        out=ev[:, cs],
                        in0=pt[:, cs],
                        in1=bias_sb[:, nb * TB + h * (TB // 2) : nb * TB + (h + 1) * (TB // 2)],
                    )
                    eng.dma_start(
                        out=out_t[:, po : po + 1, nb * TB + h * (TB // 2) : nb * TB + (h + 1) * (TB // 2)],
                        in_=ev[:, None, cs],
                    )
                return
            nc.vector.tensor_add(
                out=ev[:],
                in0=pt[:],
                in1=bias_sb[:, nb * TB : (nb + 1) * TB],
            )
            # Alternate the last block's store gens across the two HWDGE
            # rings so the final chain never queues behind a prior gen.
            eng = nc.scalar if (last_block and sub % 2 == 1) else nc.sync
            eng.dma_start(
                out=out_t[:, po : po + 1, nb * TB : (nb + 1) * TB],
                in_=ev[:, None, :],
            )

        for _ in range(reps):
            nxt = load_m(0)
            for m in range(m_tiles):
                x8_t, x16_t = nxt
                for nb in range(n_blocks):
                    if nb == 2 and m + 1 < m_tiles:
                        # Prefetch the next m-tile two n-blocks early: the
                        # loads jump the SP ring ahead of this m-tile's
                        # remaining stores (which have slack).
                        nxt = load_m(m + 1)
                    if m == 0 and nb == 0:
                        # Startup block runs k-major so every arriving k-tile
                        # chunk unlocks 4 subtiles of PE work (the operand
                        # stream is the critical path here).
                        pts = [
                            psum.tile([P, TB], mybir.dt.float32, name=f"pts_{i}", tag="ps")
                            for i in range(m_subs)
                        ]
                        for kt in range(k8_tiles):
                            for kk in range(KS // 2):
                                for sub in range(m_subs):
                                    mm_fp8(pts[sub], x8_t, nb, kt, kk, sub,
                                           start=(kt == 0 and kk == 0))
                        for kt in range(k16_tiles):
                            for s in range(KS):
                                for sub in range(m_subs):
                                    mm_fp16(pts[sub], x16_t, nb, kt, s, sub,
                                            stop=(kt == k16_tiles - 1 and s == KS - 1))
                        for sub in range(m_subs):
                            evict(pts[sub], m, nb, sub)
                        continue
                    # Steady state runs m-subtile-major: each [128, 512]
                    # output tile does its whole K reduction back-to-back, so
                    # evictions stagger (and the tail after the last matmul is
                    # one eviction + one store instead of four).
                    for sub in range(m_subs):
                        pt = psum.tile([P, TB], mybir.dt.float32, tag="ps")
                        for kt in range(k8_tiles):
                            for kk in range(KS // 2):
                                mm_fp8(pt, x8_t, nb, kt, kk, sub,
                                       start=(kt == 0 and kk == 0))
                        for kt in range(k16_tiles):
                            for s in range(KS):
                                mm_fp16(pt, x16_t, nb, kt, s, sub,
                                        stop=(kt == k16_tiles - 1 and s == KS - 1))
                        evict(pt, m, nb, sub)

    nc.compile()
    return nc


def _pack_w_nblocks(a: np.ndarray, tb: int = 512) -> np.ndarray:
    """[N, K] row-major -> [N//tb, 128, K//128, tb] with
    block[nb, p, o, j] = a[nb*tb + j, o*128 + p]; per-partition-contiguous
    [ko, tb] planes -> large DMA descriptor runs."""
    n, k = a.shape
    v = a.reshape(n // tb, tb, k // P, P)
    return np.ascontiguousarray(v.transpose(0, 3, 2, 1))


def _pack_blocks(a: np.ndarray, tb: int = 512) -> np.ndarray:
    """[F, K] row-major -> [F//tb, K//ktw, 128, ks, tb] DMA-contiguous blocks.

    block[ft, kt, p, s, j] = a[ft*tb + j, kt*ktw + s*128 + p], i.e. each
    [128, ks, tb] block is one fully-contiguous DMA source with K on the
    partition dim (a^T layout within the block)."""
    f, k = a.shape
    ktw = min(512, k)
    kts, ks = k // ktw, ktw // P
    v = a.reshape(f // tb, tb, kts, ks, P)
    return np.ascontiguousarray(v.transpose(0, 2, 4, 3, 1))


_NC_CACHE = None


def _get_nc():
    global _NC_CACHE
    if _NC_CACHE is None:
        _NC_CACHE = build_nc()
    return _NC_CACHE


def kernel(x: np.ndarray, w: np.ndarray, b: np.ndarray) -> np.ndarray:
    import ml_dtypes
    from concourse.bass_utils import run_bass_kernel_spmd

    x = np.asarray(x, dtype=np.float32)
    w = np.asarray(w, dtype=np.float32)
    b = np.asarray(b, dtype=np.float32)

    f8 = ml_dtypes.float8_e4m3
    f16 = np.float16
    s = np.sign(w)

    # Unique DMA-blocked shards (x per batch group, sign(w) per feature
    # group), packed in parallel (numpy releases the GIL on these copies).
    from concurrent.futures import ThreadPoolExecutor

    def pack_x8(bi):
        return _pack_blocks(x[bi * MC : (bi + 1) * MC, :K8].astype(f8))

    def pack_x16(bi):
        return _pack_blocks(x[bi * MC : (bi + 1) * MC, K8:].astype(f16))

    def pack_w(fi):
        # [n_blocks, P, ko, TB] fp8 of the whole sign shard.
        return _pack_w_nblocks(s[fi * NC : (fi + 1) * NC].astype(f8))

    with ThreadPoolExecutor(max_workers=8) as pool:
        x8_f = [pool.submit(pack_x8, bi) for bi in range(P_BATCH)]
        x16_f = [pool.submit(pack_x16, bi) for bi in range(P_BATCH)]
        w_f = [pool.submit(pack_w, fi) for fi in range(P_FEAT)]
        x8_shards = [f.result() for f in x8_f]
        x16_shards = [f.result() for f in x16_f]
        w_shards = [f.result() for f in w_f]
    b_shards = [np.ascontiguousarray(b[fi * NC : (fi + 1) * NC]) for fi in range(P_FEAT)]

    in_maps = []
    for c in range(N_CORES):
        bi, fi = divmod(c, P_FEAT)
        in_maps.append(
            {
                "xt8": x8_shards[bi],
                "xt16": x16_shards[bi],
                "wt": w_shards[fi],
                "bias": b_shards[fi],
            }
        )

    nc = _get_nc()
    try:
        results = run_bass_kernel_spmd(
            nc, in_maps, core_ids=list(range(N_CORES))
        ).results
    except Exception:
        # One retry for transient runtime/relay failures.
        results = run_bass_kernel_spmd(
            nc, in_maps, core_ids=list(range(N_CORES))
        ).results

    out = np.empty((M, N), dtype=np.float32)
    for c in range(N_CORES):
        bi, fi = divmod(c, P_FEAT)
        out[bi * MC : (bi + 1) * MC, fi * NC : (fi + 1) * NC] = results[c]["out"]
    return out


# revision 19
# speedup vs baseline: 1.5430x; 1.0008x over previous
"""BinaryLinear (straight-through sign(w)) kernel for Trainium2, 8 NeuronCores.

Computes out = x @ sign(w).T + b for
  x: [8192, 2048] f32, w: [4096, 2048] f32, b: [4096] f32 -> out [8192, 4096] f32.

Sharding: 4-way data parallel (batch) x 2-way tensor parallel (out_features).
Each core computes a [2048, 2048] block of the output:
  out[bi*2048:(bi+1)*2048, fi*2048:(fi+1)*2048]
    = x_shard @ sign(w_shard).T + b_shard.

Per-core device kernel — mixed-precision contraction, fp32 accumulate:
  - K is split K8 + K16.  The first K8 in_features use fp8e4 (e4m3) operands
    with perf_mode=DoubleRow (2 fp8 weights per PE cell, 256-deep virtual
    contraction): 2x the PE FLOP rate of fp16.  sign(w) in {-1,0,1} is exact
    in e4m3; only x pays quantization error (~2.65% rms per element, measured
    on the fixed problem data), diluted to 2.65%*sqrt(K8/2048) of the output.
    K8=1024 -> 1.87% rel l2, inside the 2e-2 budget.  The remaining K16
    columns run in fp16 (exact to ~3e-4).
  - the whole sign(w)^T shard ships and lives in SBUF as ONE fp8 plane
    (1 byte per weight): the DoubleRow phase reads it as fp8 pairs, and the
    fp16 phase uses it directly as the MOVING operand against fp16 x (the
    PE upconverts per-operand; cost and numerics verified on hardware).
    It loads once over the ACT HWDGE ring, n0 first in consumption-order
    chunks, so the startup block streams in lockstep with the x loads;
  - x^T tiles stream through multi-buffered pools on the SP HWDGE ring,
    with the next m-tile prefetched two n-blocks early;
  - PSUM accumulation is m-subtile-major: each [128, 512] output tile runs
    its whole K reduction back-to-back, so evictions stagger and the tail
    after the last matmul is one eviction + one store instead of four;
  - bias is added during the PSUM->SBUF copyback on the vector engine.
"""

from contextlib import ExitStack

import numpy as np

# Full problem shapes (hardcoded per the grading contract).
M, K, N = 8192, 2048, 4096
P_BATCH, P_FEAT = 4, 2  # 4 x 2 core grid
MC, NC = M // P_BATCH, N // P_FEAT  # 2048, 2048 per-core block
N_CORES = P_BATCH * P_FEAT
P = 128
K8 = 1024  # in_features contracted in fp8e4 DoubleRow (multiple of 512)
K16 = K - K8  # in_features contracted in fp16
N_WARM = 5  # HAM warmup matmuls
TAIL_SPLIT = True  # halve the final eviction+store chain


def build_nc(mc: int = MC, nc_dim: int = NC, reps: int = 1):
    """Build + compile the per-core Bass module:
    out[mc, nc_dim] = x8^T.T @ w8 + x16^T.T @ w16 + bias.

    reps > 1 repeats the whole computation (for slope-based benchmarking)."""
    import concourse.mybir as mybir
    import concourse.tile as tile
    from concourse import bacc

    TB = 512  # m/n tile width of the pre-blocked host layouts
    KT = 512  # k-tile width
    KS = KT // P  # k-subtiles per k-tile (4)
    ko8, ko16 = K8 // P, K16 // P
    k8_tiles, k16_tiles = K8 // KT, K16 // KT
    m_tiles = mc // TB
    m_subs = TB // P  # m-subtiles per m-tile (4)
    n_blocks = nc_dim // TB

    nc = bacc.Bacc("TRN2", target_bir_lowering=False, debug=False)
    DR = mybir.MatmulPerfMode.DoubleRow

    # x inputs arrive pre-blocked on the host (see _pack_blocks): each
    # [P, KS, TB] block is fully contiguous in DRAM, so every DMA has
    # large per-partition descriptor runs instead of strided ones.
    xt8 = nc.dram_tensor(
        "xt8", [m_tiles, k8_tiles, P, KS, TB], mybir.dt.float8e4,
        kind="ExternalInput",
    )
    xt16 = nc.dram_tensor(
        "xt16", [m_tiles, k16_tiles, P, KS, TB], mybir.dt.float16,
        kind="ExternalInput",
    )
    # w uses an n-block-major layout ([nb, p, ko, n]) so each n-block's
    # preload is contiguous per partition on BOTH sides.  The whole sign
    # matrix ships as ONE fp8 tensor: the DoubleRow phase needs fp8 anyway,
    # and the fp16 phase uses the fp8 plane directly as its MOVING operand
    # (sign values are exact in e4m3; the PE upconverts per-operand, and the
    # matmul cost is set by the moving dtype at the same 1 row/cycle as
    # fp16) — 1 byte/weight instead of 2 through the startup DMA crunch.
    ko = ko8 + ko16
    wt = nc.dram_tensor(
        "wt", [n_blocks, P, ko, TB], mybir.dt.float8e4, kind="ExternalInput"
    )
    bias = nc.dram_tensor("bias", [nc_dim], mybir.dt.float32, kind="ExternalInput")
    out = nc.dram_tensor("out", [mc, nc_dim], mybir.dt.float32, kind="ExternalOutput")

    with tile.TileContext(nc) as tc, ExitStack() as ctx:
        # HAM warmup: the PE clock is gated (0.65/1.2 GHz) until ~3 us of
        # sustained activity. Start throwaway matmuls as early as possible
        # (gpsimd memset is ~100 ns) so the ramp burns while the first
        # operand DMAs are in flight. The scratch SBUF pool stays OPEN so
        # its slot is never reused; only the PSUM bank is returned.
        warm_sb = ctx.enter_context(tc.tile_pool(name="warm_sb", bufs=1))
        scratch = warm_sb.tile([P, 512], mybir.dt.float16)
        nc.vector.memset(scratch[:], 0.0)
        with tc.tile_pool(name="warm_ps", bufs=1, space="PSUM") as wps_pool:
            ps = wps_pool.tile([P, 512], mybir.dt.float32)
            for _ in range(N_WARM):
                nc.tensor.matmul(
                    ps[:], scratch[:, :P], scratch[:], start=True, stop=True
                )

        const = ctx.enter_context(tc.tile_pool(name="const", bufs=1))
        x8_pool = ctx.enter_context(tc.tile_pool(name="x8", bufs=2 * k8_tiles))
        x16_pool = ctx.enter_context(tc.tile_pool(name="x16", bufs=2 * k16_tiles))
        ev_pool = ctx.enter_context(tc.tile_pool(name="ev", bufs=4))
        psum = ctx.enter_context(tc.tile_pool(name="psum", bufs=6, space="PSUM"))

        # Whole sign(w)^T shard resident in SBUF, n-block-major, as an fp8
        # plane (first K8 in_features) and an fp16 plane (rest):
        #   w8_sb[p, nb, o, j]  = sign(w)^T[o*128 + p, nb*TB + j],  o <  ko8
        #   w16_sb[p, nb, o, j] = sign(w)^T[K8 + o*128 + p, nb*TB + j]
        w_sb = const.tile([P, n_blocks, ko, TB], mybir.dt.float8e4)

        bias_sb = const.tile([P, nc_dim], mybir.dt.float32)

        # The cost model serializes ALL DMA transfers on one shared pipe and
        # alternates HWDGE descriptor generation between the SP and ACT
        # queues; in-queue gens are FIFO, so transfer order tracks issue
        # order. The early transfer order is the startup critical path: n0
        # split per k-tile (its chunks interleave with the SP-ring x loads
        # of m0, matching the first block's consumption order), then the
        # bias row, then n1..n3 whole. SWDGE is avoided for w entirely — its
        # independent transfers would jump ahead of the critical n0 stream.
        # n0 in 4 k-tile chunks (matching the startup block's consumption
        # order against the SP-ring x loads), bias, then n1..n3 in
        # fp8-phase/fp16-phase halves so each matmul group's region dep
        # waits only on the half it reads.
        for kt in range(k8_tiles):
            sl = slice(kt * KS, (kt + 1) * KS)
            nc.scalar.dma_start(out=w_sb[:, 0, sl, :], in_=wt.ap()[0, :, sl, :])
        nc.scalar.dma_start(out=bias_sb[:1, :], in_=bias.ap()[None, :])
        for h in range(2 * k16_tiles):
            sl = slice(ko8 + h * (KS // 2), ko8 + (h + 1) * (KS // 2))
            nc.scalar.dma_start(out=w_sb[:, 0, sl, :], in_=wt.ap()[0, :, sl, :])
        for nb in range(1, n_blocks):
            nc.scalar.dma_start(out=w_sb[:, nb, :ko8], in_=wt.ap()[nb, :, :ko8])
            nc.scalar.dma_start(out=w_sb[:, nb, ko8:], in_=wt.ap()[nb, :, ko8:])

        nc.gpsimd.partition_broadcast(bias_sb[:], bias_sb[:1, :])

        out_t = out.ap().rearrange("(o p) n -> p o n", p=P)

        def load_m(m, split16=False):
            """Issue the x-tile loads for m-tile m on the SP HWDGE ring.
            split16 halves the fp16 loads so the startup block's first fp16
            matmuls unblock a k-subtile-pair earlier."""
            t8 = []
            for kt in range(k8_tiles):
                t = x8_pool.tile([P, KS, TB], mybir.dt.float8e4, tag="x8")
                nc.sync.dma_start(out=t[:], in_=xt8.ap()[m, kt])
                t8.append(t)
            t16 = []
            for kt in range(k16_tiles):
                t = x16_pool.tile([P, KS, TB], mybir.dt.float16, tag="x16")
                if split16:
                    h = KS // 2
                    nc.sync.dma_start(out=t[:, :h], in_=xt16.ap()[m, kt, :, :h])
                    nc.sync.dma_start(out=t[:, h:], in_=xt16.ap()[m, kt, :, h:])
                else:
                    nc.sync.dma_start(out=t[:], in_=xt16.ap()[m, kt])
                t16.append(t)
            return t8, t16

        def mm_fp8(pt, x8_t, nb, kt, kk, sub, start):
            ms = slice(sub * P, (sub + 1) * P)
            nc.tensor.matmul(
                pt[:],
                x8_t[kt][:, 2 * kk : 2 * kk + 2, ms],
                w_sb[:, nb, kt * KS + 2 * kk : kt * KS + 2 * kk + 2, :],
                start=start,
                stop=False,
                perf_mode=DR,
            )

        def mm_fp16(pt, x16_t, nb, kt, s, sub, stop):
            ms = slice(sub * P, (sub + 1) * P)
            nc.tensor.matmul(
                pt[:],
                x16_t[kt][:, s, ms],
                w_sb[:, nb, ko8 + kt * KS + s, :],
                start=False,
                stop=stop,
            )

        def evict(pt, m, nb, sub):
            po = m * m_subs + sub
            last_block = m == m_tiles - 1 and nb == n_blocks - 1
            ev = ev_pool.tile([P, TB], mybir.dt.float32, tag="ev")
            if TAIL_SPLIT and last_block and sub == m_subs - 1:
                # Final tile: halve the add+store chain so the last HBM
                # write (whose completion latency ends the kernel) starts as
                # early as possible; the two store gens go to different
                # HWDGE rings so they don't serialize.
                for h, eng in ((0, nc.sync), (1, nc.scalar)):
                    cs = slice(h * (TB // 2), (h + 1) * (TB // 2))
                    nc.vector.tensor_add(
                        out=ev[:, cs],
                        in0=pt[:, cs],
                        in1=bias_sb[:, nb * TB + h * (TB // 2) : nb * TB + (h + 1) * (TB // 2)],
                    )
                    eng.dma_start(
                        out=out_t[:, po : po + 1, nb * TB + h * (TB // 2) : nb * TB + (h + 1) * (TB // 2)],
                        in_=ev[:, None, cs],
                    )
                return
            nc.vector.tensor_add(
                out=ev[:],
                in0=pt[:],
                in1=bias_sb[:, nb * TB : (nb + 1) * TB],
            )
            # Alternate the last block's store gens across the two HWDGE
            # rings so the final chain never queues behind a prior gen.
            eng = nc.scalar if (last_block and sub % 2 == 1) else nc.sync
            eng.dma_start(
                out=out_t[:, po : po + 1, nb * TB : (nb + 1) * TB],
                in_=ev[:, None, :],
            )

        for _ in range(reps):
            nxt = load_m(0, split16=True)
            for m in range(m_tiles):
                x8_t, x16_t = nxt
                for nb in range(n_blocks):
                    if nb == 2 and m + 1 < m_tiles:
                        # Prefetch the next m-tile two n-blocks early: the
                        # loads jump the SP ring ahead of this m-tile's
                        # remaining stores (which have slack).
                        nxt = load_m(m + 1)
                    if m == 0 and nb == 0:
                        # Startup block runs k-major so every arriving k-tile
                        # chunk unlocks 4 subtiles of PE work (the operand
                        # stream is the critical path here).
                        pts = [
                            psum.tile([P, TB], mybir.dt.float32, name=f"pts_{i}", tag="ps")
                            for i in range(m_subs)
                        ]
                        for kt in range(k8_tiles):
                            for kk in range(KS // 2):
                                for sub in range(m_subs):
                                    mm_fp8(pts[sub], x8_t, nb, kt, kk, sub,
                                           start=(kt == 0 and kk == 0))
                        for kt in range(k16_tiles):
                            for s in range(KS):
                                for sub in range(m_subs):
                                    mm_fp16(pts[sub], x16_t, nb, kt, s, sub,
                                            stop=(kt == k16_tiles - 1 and s == KS - 1))
                        for sub in range(m_subs):
                            evict(pts[sub], m, nb, sub)
                        continue
                    # Steady state runs m-subtile-major: each [128, 512]
                    # output tile does its whole K reduction back-to-back, so
                    # evictions stagger (and the tail after the last matmul is
                    # one eviction + one store instead of four).
                    for sub in range(m_subs):
                        pt = psum.tile([P, TB], mybir.dt.float32, tag="ps")
                        for kt in range(k8_tiles):
                            for kk in range(KS // 2):
                                mm_fp8(pt, x8_t, nb, kt, kk, sub,
                                       start=(kt == 0 and kk == 0))
                        for kt in range(k16_tiles):
                            for s in range(KS):
                                mm_fp16(pt, x16_t, nb, kt, s, sub,
                                        stop=(kt == k16_tiles - 1 and s == KS - 1))
                        evict(pt, m, nb, sub)

    nc.compile()
    return nc


def _pack_w_nblocks(a: np.ndarray, tb: int = 512) -> np.ndarray:
    """[N, K] row-major -> [N//tb, 128, K//128, tb] with
    block[nb, p, o, j] = a[nb*tb + j, o*128 + p]; per-partition-contiguous
    [ko, tb] planes -> large DMA descriptor runs."""
    n, k = a.shape
    v = a.reshape(n // tb, tb, k // P, P)
    return np.ascontiguousarray(v.transpose(0, 3, 2, 1))


def _pack_blocks(a: np.ndarray, tb: int = 512) -> np.ndarray:
    """[F, K] row-major -> [F//tb, K//ktw, 128, ks, tb] DMA-contiguous blocks.

    block[ft, kt, p, s, j] = a[ft*tb + j, kt*ktw + s*128 + p], i.e. each
    [128, ks, tb] block is one fully-contiguous DMA source with K on the
    partition dim (a^T layout within the block)."""
    f, k = a.shape
    ktw = min(512, k)
    kts, ks = k // ktw, ktw // P
    v = a.reshape(f // tb, tb, kts, ks, P)
    return np.ascontiguousarray(v.transpose(0, 2, 4, 3, 1))


_NC_CACHE = None


def _get_nc():
    global _NC_CACHE
    if _NC_CACHE is None:
        _NC_CACHE = build_nc()
    return _NC_CACHE


def kernel(x: np.ndarray, w: np.ndarray, b: np.ndarray) -> np.ndarray:
    import ml_dtypes
    from concourse.bass_utils import run_bass_kernel_spmd

    x = np.asarray(x, dtype=np.float32)
    w = np.asarray(w, dtype=np.float32)
    b = np.asarray(b, dtype=np.float32)

    f8 = ml_dtypes.float8_e4m3
    f16 = np.float16
    s = np.sign(w)

    # Unique DMA-blocked shards (x per batch group, sign(w) per feature
    # group), packed in parallel (numpy releases the GIL on these copies).
    from concurrent.futures import ThreadPoolExecutor

    def pack_x8(bi):
        return _pack_blocks(x[bi * MC : (bi + 1) * MC, :K8].astype(f8))

    def pack_x16(bi):
        return _pack_blocks(x[bi * MC : (bi + 1) * MC, K8:].astype(f16))

    def pack_w(fi):
        # [n_blocks, P, ko, TB] fp8 of the whole sign shard.
        return _pack_w_nblocks(s[fi * NC : (fi + 1) * NC].astype(f8))

    with ThreadPoolExecutor(max_workers=8) as pool:
        x8_f = [pool.submit(pack_x8, bi) for bi in range(P_BATCH)]
        x16_f = [pool.submit(pack_x16, bi) for bi in range(P_BATCH)]
        w_f = [pool.submit(pack_w, fi) for fi in range(P_FEAT)]
        x8_shards = [f.result() for f in x8_f]
        x16_shards = [f.result() for f in x16_f]
        w_shards = [f.result() for f in w_f]
    b_shards = [np.ascontiguousarray(b[fi * NC : (fi + 1) * NC]) for fi in range(P_FEAT)]

    in_maps = []
    for c in range(N_CORES):
        bi, fi = divmod(c, P_FEAT)
        in_maps.append(
            {
                "xt8": x8_shards[bi],
                "xt16": x16_shards[bi],
                "wt": w_shards[fi],
                "bias": b_shards[fi],
            }
        )

    nc = _get_nc()
    try:
        results = run_bass_kernel_spmd(
            nc, in_maps, core_ids=list(range(N_CORES))
        ).results
    except Exception:
        # One retry for transient runtime/relay failures.
        results = run_bass_kernel_spmd(
            nc, in_maps, core_ids=list(range(N_CORES))
        ).results

    out = np.empty((M, N), dtype=np.float32)
    for c in range(N_CORES):
        bi, fi = divmod(c, P_FEAT)
        out[bi * MC : (bi + 1) * MC, fi * NC : (fi + 1) * NC] = results[c]["out"]
    return out
